# revision 1
# baseline (speedup 1.0000x reference)
"""Cumulative-probability head on 8 Trainium2 NeuronCores.

out[b, j] = sum_{i<=j} relu(x @ W_h^T + b_h)[b, i] + (x @ W_base^T + b_base)[b]

Data-parallel: x is sharded along batch (1024 rows per core); the small
weights are replicated. The host passes x pre-transposed per shard
([D, 1024], contiguous rows) so the contraction dim lands on SBUF
partitions with no on-device transposes. Per core:

  1. haz = xT.T @ WT_aug in float32r (FP22 multiplies, fp32 PSUM accum).
     WT_aug is [2049, 516]: hazard cols 0..511, base col 512, 3 zero pad
     cols; row 2048 is the bias row, added post-accumulation on DVE
     via a partition-broadcast read. The 516 output cols split into two
     even N=258 PSUM banks (fp32r requires an even moving dim).
  2. Each 128-row batch tile accumulates the full K=2048 contraction
     in a pair of PSUM banks; four tiles are in flight (8 banks), so
     the PE has work while input chunks stream in.
  3. Input DMAs are spread over three queue rings (Sync HWDGE, Scalar
     HWDGE, GPSIMD SWDGE) in k-order — one ring caps at ~160 GB/s,
     which would serialize the 12 MB of per-core input.
  4. ReLU on ScalarE (hazard cols only; base col stays unactivated),
     then the inclusive cumsum along T on DVE via tensor_tensor_scan
     with the base hazard as the per-partition initial state.
"""

import numpy as np

import concourse.bass as bass
import concourse.tile as tile
from concourse import bacc, mybir
from concourse.bass_utils import run_bass_kernel_spmd

B, D, T = 8192, 2048, 512
NCORES = 8
BLOC = B // NCORES            # 1024 rows per core
NB = BLOC // 128              # 8 batch tiles per core
NK = D // 128                 # 16 contraction chunks
TP = 516                      # padded output width (512 hazard + base + 3 junk)
NA = 258                      # output cols 0..257 in PSUM bank A
NBK = TP - NA                 # cols 258..515 in bank B (col 512 = base)
BOFF = T - NA                 # offset of the base col inside bank B (254)

F32 = mybir.dt.float32
F32R = mybir.dt.float32r


def _build_program():
    nc = bacc.Bacc("TRN2", target_bir_lowering=False, debug=False)

    xt_d = nc.dram_tensor("xt", [D, BLOC], F32R, kind="ExternalInput")
    wt_d = nc.dram_tensor("wt", [D + 1, TP], F32R, kind="ExternalInput")
    out_d = nc.dram_tensor("out", [BLOC, T], F32, kind="ExternalOutput")

    with tile.TileContext(nc) as tc:
        with (
            tc.tile_pool(name="consts", bufs=1) as consts,
            tc.tile_pool(name="wt", bufs=1) as wtp,
            tc.tile_pool(name="xt", bufs=1) as xtp,
            tc.tile_pool(name="haz", bufs=4) as hazp,
            tc.tile_pool(name="outp", bufs=4) as outp,
            tc.tile_pool(name="ps_mm", bufs=4, space="PSUM") as ps_mm,
        ):
            zeros = consts.tile([128, T], F32)
            nc.vector.memset(zeros, 0.0)

            # Input loads, k-ordered, spread over three DMA rings. The two
            # HWDGE rings (Sync, Scalar) are faster than the GPSIMD SWDGE
            # ring, so the first chunks — which gate the PE pipeline ramp —
            # go to the HWDGE rings, and the slow ring only carries late
            # chunks. Weights (half the size of an x chunk) ride opposite
            # rings from their x chunk so the pair lands together.
            XT_RING = [0, 1, 0, 1, 2, 0, 1, 2, 0, 1, 2, 0, 1, 2, 0, 1]
            WT_RING = [1, 0, 1, 0, 0, 1, 2, 0, 1, 2, 0, 1, 2, 0, 1, 2]
            rings = [nc.sync, nc.scalar, nc.gpsimd]
            xt_tiles = []
            wt_tiles = []
            wbias_bc = wtp.tile([128, TP], F32, tag="wbias")
            H = BLOC // 2
            for k in range(NK):
                # x chunks load in column halves: the first four (live)
                # batch tiles only read cols 0..511, so streaming ALL
                # first-halves before any second-half lets them retire on
                # half the input volume and frees PSUM banks mid-window
                # for batch tiles 4..7.
                xk = xtp.tile([128, BLOC], F32R, tag=f"xt{k}")
                rings[XT_RING[k]].dma_start(
                    out=xk[:, 0:H], in_=xt_d[128 * k : 128 * (k + 1), 0:H]
                )
                xt_tiles.append(xk)
                w = wtp.tile([128, TP], F32R, tag=f"wt{k}")
                rings[WT_RING[k]].dma_start(out=w, in_=wt_d[128 * k : 128 * (k + 1), :])
                wt_tiles.append(w)
                if k == 8:
                    # Bias row replicated across all 128 partitions with a
                    # partition-stride-0 DMA read (engines can't read
                    # stride-0 partition APs, but DMA can). Emitted mid-
                    # stream on the GPSIMD ring: early enough to be resident
                    # long before the first b-tile's bias add (which gates
                    # PSUM bank recycling), late enough not to delay the
                    # chunks that pace the PE ramp.
                    bias_src = wt_d[D : D + 1, :]
                    nc.gpsimd.dma_start(
                        out=wbias_bc,
                        in_=bass.AP(
                            tensor=bias_src.tensor,
                            offset=bias_src.offset,
                            ap=[[0, 128]] + list(bias_src.ap[1:]),
                        ).bitcast(F32),
                    )
            for k in range(NK):
                rings[XT_RING[k]].dma_start(
                    out=xt_tiles[k][:, H:BLOC],
                    in_=xt_d[128 * k : 128 * (k + 1), H:BLOC],
                )

            # Full-K accumulation per 128-row batch tile; bufs=4 on each
            # PSUM tag -> 4 b-tiles in flight across all 8 banks.
            for b in range(NB):
                pA = ps_mm.tile([128, NA], F32, tag="pA")
                pB = ps_mm.tile([128, NBK], F32, tag="pB")
                for k in range(NK):
                    xt_ap = xt_tiles[k][:, 128 * b : 128 * (b + 1)]
                    w = wt_tiles[k]
                    nc.tensor.matmul(
                        pA[:], xt_ap, w[:, 0:NA],
                        start=(k == 0), stop=(k == NK - 1),
                    )
                    nc.tensor.matmul(
                        pB[:], xt_ap, w[:, NA:TP],
                        start=(k == 0), stop=(k == NK - 1),
                    )

                # Bias row added on DVE via a partition-broadcast read —
                # keeps the K=1 ones-row matmuls (and their LDWEIGHTS)
                # off the PE stream.
                pre = hazp.tile([128, 2, NA], F32, tag="pre")
                nc.vector.tensor_add(pre[:, 0, :], pA[:], wbias_bc[:, 0:NA])
                nc.vector.tensor_add(pre[:, 1, :], pB[:], wbias_bc[:, NA:TP])

                haz = hazp.tile([128, T], F32, tag="haz")
                base = hazp.tile([128, 1], F32, tag="base")
                nc.scalar.activation(
                    out=haz[:, 0:NA], in_=pre[:, 0, :],
                    func=mybir.ActivationFunctionType.Relu,
                )
                nc.scalar.activation(
                    out=haz[:, NA:T], in_=pre[:, 1, 0:BOFF],
                    func=mybir.ActivationFunctionType.Relu,
                )
                nc.scalar.copy(out=base, in_=pre[:, 1, BOFF : BOFF + 1])

                cum = outp.tile([128, T], F32)
                nc.vector.tensor_tensor_scan(
                    out=cum,
                    data0=haz,
                    data1=zeros,
                    initial=base,
                    op0=mybir.AluOpType.add,
                    op1=mybir.AluOpType.add,
                )
                nc.scalar.dma_start(out=out_d[128 * b : 128 * (b + 1), :], in_=cum)

    nc.compile()
    return nc


_NC_CACHE = None


def kernel(x, W_hazard, b_hazard, W_base, b_base):
    global _NC_CACHE
    if _NC_CACHE is None:
        _NC_CACHE = _build_program()
    nc = _NC_CACHE

    x = np.asarray(x, dtype=np.float32)
    W_cat = np.concatenate(
        [np.asarray(W_hazard, np.float32), np.asarray(W_base, np.float32)], axis=0
    )  # [513, 2048]
    bias_row = np.concatenate(
        [np.asarray(b_hazard, np.float32), np.asarray(b_base, np.float32)]
    )  # [513]
    wt = np.concatenate([W_cat.T, bias_row[None, :]], axis=0)  # [2049, 513]
    wt = np.ascontiguousarray(
        np.concatenate([wt, np.zeros((D + 1, TP - (T + 1)), np.float32)], axis=1)
    )  # [2049, 516]

    in_maps = [
        {
            "xt": np.ascontiguousarray(x[BLOC * i : BLOC * (i + 1)].T),
            "wt": wt,
        }
        for i in range(NCORES)
    ]
    res = run_bass_kernel_spmd(nc, in_maps, list(range(NCORES)))
    return np.concatenate([res.results[i]["out"] for i in range(NCORES)], axis=0)



# revision 3
# speedup vs baseline: 1.6057x; 1.6057x over previous
"""Cumulative-probability head on 8 Trainium2 NeuronCores.

out[b, j] = sum_{i<=j} relu(x @ W_h^T + b_h)[b, i] + (x @ W_base^T + b_base)[b]

Data-parallel: x sharded along batch (1024 rows/core), weights replicated.

Per-core strategy (fp8 DoubleRow):
  - x and W are quantized host-side to TRN fp8-e4m3 (ml_dtypes.float8_e4m3,
    matching TRN FP8_EXP4: max normal 240) with power-of-2 scales
    Sx=16, Sw=512. The matmul runs in MatmulPerfMode.DoubleRow (2 fp8
    MACs/cell/cycle -> 157 TF/s), accumulating S*x@W in fp32 PSUM.
  - Contraction 2048 = 8 chunks x (128 partitions x 2 doublerow slots):
    k = 256*c + 2*p + i. Tiles are [128, 2, N]; lhsT = x chunk (stationary,
    batch on free dim), rhs = W chunk (moving, T on free dim).
  - Hazard matmul N=512 fills exactly one PSUM bank; the base column rides
    as a tiny N=2 matmul into a shared [128,16] PSUM tile (one bank, one
    2-col accumulation region per 128-row batch tile).
  - Batch processed in 2 waves of 512 rows (4 b-tiles each), chunk-outer
    loop so early chunks feed the PE while later chunks stream in.
    PSUM: 6 hazard banks (ring) + 1 base bank.
  - Post per b-tile: DVE adds S*bias into PSUM in place, ScalarE applies
    Relu with scale 1/S into bf16, base col gets Identity(scale)+b_base,
    DVE tensor_tensor_scan (fp32 internal state) does the inclusive
    cumsum with the base as initial state, bf16 output DMA'd out.
  - Input DMAs spread over Sync/Scalar HWDGE + GPSIMD SWDGE rings,
    k-ordered; wave-1 x streams during wave-0 compute.
"""

import numpy as np
import ml_dtypes

import concourse.bass as bass
import concourse.tile as tile
from concourse import bacc, mybir
from concourse.bass_utils import run_bass_kernel_spmd

B, D, T = 8192, 2048, 512
NCORES = 8
BLOC = B // NCORES            # 1024 rows per core
WB = BLOC // 2                # 512 rows per wave
NBW = WB // 128               # 4 b-tiles per wave
NCH = D // 256                # 8 contraction chunks (256 = 128 x 2 doublerow)
TP = 516                      # padded W width: 512 hazard + base + 3 zero
SX = 16.0                     # x fp8 scale
SW = 512.0                    # W fp8 scale
S = SX * SW

F32 = mybir.dt.float32
BF16 = mybir.dt.bfloat16
F8 = mybir.dt.float8e4

F8NP = ml_dtypes.float8_e4m3
BF16NP = ml_dtypes.bfloat16


def _build_program():
    nc = bacc.Bacc("TRN2", target_bir_lowering=False, debug=False)

    xt_d = nc.dram_tensor("xt", [2, D, WB], F8, kind="ExternalInput")
    wt_d = nc.dram_tensor("wt", [D, TP], F8, kind="ExternalInput")
    bias_d = nc.dram_tensor("bias", [1, TP], BF16, kind="ExternalInput")
    out_d = nc.dram_tensor("out", [BLOC, T], BF16, kind="ExternalOutput")

    DR = mybir.MatmulPerfMode.DoubleRow
    Relu = mybir.ActivationFunctionType.Relu
    Ident = mybir.ActivationFunctionType.Identity

    with tile.TileContext(nc) as tc:
        with (
            tc.tile_pool(name="consts", bufs=1) as consts,
            tc.tile_pool(name="wt", bufs=1) as wtp,
            tc.tile_pool(name="xt", bufs=1) as xtp,
            tc.tile_pool(name="haz", bufs=4) as hazp,
            tc.tile_pool(name="outp", bufs=4) as outp,
            tc.tile_pool(name="ps", bufs=6, space="PSUM") as psp,
            tc.tile_pool(name="psb", bufs=1, space="PSUM") as psbp,
        ):
            zeros = consts.tile([128, T], BF16, tag="zeros")
            nc.vector.memset(zeros, 0.0)
            bias_bc = consts.tile([128, TP], BF16, tag="bias")

            rings = [nc.sync, nc.scalar, nc.gpsimd]
            # Ring choice per transfer: HWDGE rings (sync=0, scalar=1)
            # carry the chunks that gate the PE ramp; SWDGE (2) takes
            # late-need traffic.
            WT_RING = [0, 2, 1, 0, 2, 1, 0, 2]
            X0_RING = [1, 0, 2, 1, 0, 2, 1, 0]
            X1_RING = [1, 0, 2, 1, 0, 2, 1, 0]

            wt_tiles = []
            xt_tiles = [[None] * NCH for _ in range(2)]
            for c in range(NCH):
                w = wtp.tile([128, 2, TP], F8, tag=f"wt{c}")
                rings[WT_RING[c]].dma_start(
                    out=w, in_=wt_d[256 * c : 256 * (c + 1), :]
                )
                wt_tiles.append(w)
                xk = xtp.tile([128, 2, WB], F8, tag=f"x0_{c}")
                rings[X0_RING[c]].dma_start(
                    out=xk, in_=xt_d[0, 256 * c : 256 * (c + 1), :]
                )
                xt_tiles[0][c] = xk
            # Bias row broadcast to 128 partitions via stride-0 partition
            # DMA read (engines can't read stride-0 partition APs; DMA can).
            bsrc = bias_d[0:1, :]
            nc.gpsimd.dma_start(
                out=bias_bc,
                in_=bass.AP(
                    tensor=bsrc.tensor,
                    offset=bsrc.offset,
                    ap=[[0, 128]] + list(bsrc.ap[1:]),
                ),
            )
            for c in range(NCH):
                xk = xtp.tile([128, 2, WB], F8, tag=f"x1_{c}")
                rings[X1_RING[c]].dma_start(
                    out=xk, in_=xt_d[1, 256 * c : 256 * (c + 1), :]
                )
                xt_tiles[1][c] = xk

            base_ps = psbp.tile([128, 2 * NCORES], F32, tag="bps")
            out_rings = [nc.scalar, nc.sync]

            for wv in range(2):
                ps_tiles = [
                    psp.tile([128, T], F32, tag="ps", name=f"ps_{wv}_{i}")
                    for i in range(NBW)
                ]
                for c in range(NCH):
                    for bl in range(NBW):
                        b = NBW * wv + bl
                        lhsT = xt_tiles[wv][c][:, :, 128 * bl : 128 * (bl + 1)]
                        nc.tensor.matmul(
                            ps_tiles[bl][:],
                            lhsT,
                            wt_tiles[c][:, :, 0:T],
                            start=(c == 0),
                            stop=(c == NCH - 1),
                            perf_mode=DR,
                        )
                        nc.tensor.matmul(
                            base_ps[:, 2 * b : 2 * b + 2],
                            lhsT,
                            wt_tiles[c][:, :, T : T + 2],
                            start=(c == 0),
                            stop=(c == NCH - 1),
                            perf_mode=DR,
                        )
                for bl in range(NBW):
                    b = NBW * wv + bl
                    ps = ps_tiles[bl]
                    # psum += S*bias (in place, frees no SBUF tile)
                    nc.vector.tensor_add(ps[:], ps[:], bias_bc[:, 0:T])
                    haz = hazp.tile([128, T], BF16, tag="haz")
                    nc.scalar.activation(out=haz, in_=ps[:], func=Relu, scale=1.0 / S)
                    baset = hazp.tile([128, 1], BF16, tag="base")
                    nc.scalar.activation(
                        out=baset,
                        in_=base_ps[:, 2 * b : 2 * b + 1],
                        func=Ident,
                        scale=1.0 / S,
                        bias=bias_bc[:, T : T + 1],
                    )
                    cum = outp.tile([128, T], BF16, tag="cum")
                    nc.vector.tensor_tensor_scan(
                        out=cum,
                        data0=haz,
                        data1=zeros,
                        initial=baset,
                        op0=mybir.AluOpType.add,
                        op1=mybir.AluOpType.add,
                    )
                    out_rings[b % 2].dma_start(
                        out=out_d[128 * b : 128 * (b + 1), :], in_=cum
                    )

    nc.compile()
    return nc


_NC_CACHE = None


def prep_in_maps(x, W_hazard, b_hazard, W_base, b_base):
    x = np.asarray(x, np.float32)
    Wh = np.asarray(W_hazard, np.float32)
    bh = np.asarray(b_hazard, np.float32)
    Wb = np.asarray(W_base, np.float32).reshape(1, D)
    bb = np.asarray(b_base, np.float32).reshape(1)

    wt = np.zeros((D, TP), np.float32)
    wt[:, 0 : T + 1] = np.concatenate([Wh, Wb], axis=0).T * SW
    np.clip(wt, -240.0, 240.0, out=wt)
    wt8 = wt.astype(F8NP)

    bias = np.zeros((1, TP), np.float32)
    bias[0, 0:T] = bh * S
    bias[0, T] = bb[0]
    bias16 = bias.astype(BF16NP)

    x8 = np.clip(x * SX, -240.0, 240.0).astype(F8NP)  # [B, D]
    in_maps = []
    for i in range(NCORES):
        xs = x8[BLOC * i : BLOC * (i + 1)]  # [1024, D]
        xt = np.ascontiguousarray(xs.T.reshape(D, 2, WB).transpose(1, 0, 2))
        in_maps.append({"xt": xt, "wt": wt8, "bias": bias16})
    return in_maps


def kernel(x, W_hazard, b_hazard, W_base, b_base):
    global _NC_CACHE
    if _NC_CACHE is None:
        _NC_CACHE = _build_program()
    in_maps = prep_in_maps(x, W_hazard, b_hazard, W_base, b_base)
    res = run_bass_kernel_spmd(_NC_CACHE, in_maps, list(range(NCORES)))
    return np.concatenate(
        [res.results[i]["out"].astype(np.float32) for i in range(NCORES)], axis=0
    )
